# revision 1
# baseline (speedup 1.0000x reference)
"""Trainium2 Bass kernel for DifferentiableGMM log-likelihood.

Computes  out[n] = logsumexp_k( -0.5*||(x[n]-mu[k])/s[k]||^2 - log|s[k]| + log w[k] )
for N=2,000,000 points, K=16 diagonal-covariance components, D=3.

Strategy (pure data parallel over 8 cores, 262144 padded points per core):
  The per-component Gaussian log-prob is a quadratic in x:
      lp[n,k] = sum_d A[k,d]*x[n,d]^2 + B[k,d]*x[n,d] + c_k
  so it is computed as a matmul of per-point features F = [x^2, x] against a
  block-diagonal weight matrix (8 point-groups x 16 components per matmul,
  fp32r streaming).  exp() folds c_k via the per-partition activation bias,
  the sum over k is a second (ones) matmul accumulating 16 rounds into one
  dense [128,512] PSUM tile, followed by a single Ln pass.  The device writes
  results in an interleaved order; the host gathers them back (free).
"""

import os
import numpy as np

K = 16
D = 3
EPS = 1e-6
N_CORES = 8
N_FULL = 2_000_000

# per-core tiling
T_TILES = 4                      # x-tiles per core
TILE_PTS = 128 * 512             # points per x-tile
NPC = T_TILES * TILE_PTS         # 262144 points per core
N_PAD = N_CORES * NPC            # 2097152

_compiled_cache = {}


def _build_nc(use_f32r=True):
    ablate = set(os.environ.get("GMM_ABLATE", "").split(","))
    # Force the ACT-table chooser to use the one set that holds Exp, Ln AND
    # Copy together, so no table reloads happen mid-kernel.  Other sets are
    # blanked (positions preserved: set ids index act_info.json).
    import concourse.bacc as _bacc_mod
    from concourse.hw_specs import get_activation_tables as _orig_gat
    def _only_combined(arch, __orig=_orig_gat):
        return {name: (fns if name == "natural_log_exp_and_others" else set())
                for name, fns in __orig(arch).items()}
    _bacc_mod.get_activation_tables = _only_combined
    defer_log = bool(int(os.environ.get("GMM_DEFER_LOG", "0")))
    reps = int(os.environ.get("GMM_REPS", "1"))
    tp_bufs = int(os.environ.get("GMM_TP", "3"))
    mp_bufs = int(os.environ.get("GMM_MP", "2"))
    sp_bufs = int(os.environ.get("GMM_SP", "1"))
    import concourse.bacc as bacc
    import concourse.mybir as mybir
    import concourse.tile as tile
    from concourse._compat import get_trn_type

    f32 = mybir.dt.float32
    f32r = mybir.dt.float32r
    AF = mybir.ActivationFunctionType

    mdt = f32r if use_f32r else f32

    nc = bacc.Bacc(
        get_trn_type() or "TRN2",
        target_bir_lowering=False,
        debug=False,
        num_devices=N_CORES,
    )

    x_dram = nc.dram_tensor("x", [NPC, D], f32, kind="ExternalInput")
    wdiag_dram = nc.dram_tensor("wdiag", [128, 128], mdt, kind="ExternalInput")
    cvec_dram = nc.dram_tensor("cvec", [128, 1], f32, kind="ExternalInput")
    ones_dram = nc.dram_tensor("onesbig", [128, 256], mdt, kind="ExternalInput")
    ident_dram = nc.dram_tensor("ident", [128, 128], mdt, kind="ExternalInput")
    out_dram = nc.dram_tensor("out", [NPC], f32, kind="ExternalOutput")

    with tile.TileContext(nc) as tc:
        with (
            tc.tile_pool(name="singles", bufs=1) as singles,
            tc.tile_pool(name="xin", bufs=4) as xin_pool,
            tc.tile_pool(name="fbig", bufs=4) as f_pool,
            tc.tile_pool(name="ft", bufs=int(os.environ.get("GMM_FT", "3"))) as ft_pool,
            tc.tile_pool(name="etile", bufs=int(os.environ.get("GMM_E", "3"))) as e_pool,
            tc.tile_pool(name="osb", bufs=4) as out_pool,
            tc.tile_pool(name="tpsum", bufs=tp_bufs, space="PSUM") as tpsum_pool,
            tc.tile_pool(name="mpsum", bufs=mp_bufs, space="PSUM") as mpsum_pool,
            tc.tile_pool(name="spsum", bufs=sp_bufs, space="PSUM") as spsum_pool,
        ):
            # Constants precomputed on host.  Staged through compute-engine
            # copies so consumers' waits merge into their existing DVE/ACT
            # sem domains (matmul structs allow only ONE sync wait).
            Wdiag_st = singles.tile([128, 128], mdt)
            cvec_st = singles.tile([128, 1], f32)
            ones_st = singles.tile([128, 256], mdt)
            ident_st = singles.tile([128, 128], mdt)
            nc.sync.dma_start(Wdiag_st[:], wdiag_dram[:, :])
            nc.sync.dma_start(cvec_st[:], cvec_dram[:, :])
            nc.sync.dma_start(ones_st[:], ones_dram[:, :])
            nc.sync.dma_start(ident_st[:], ident_dram[:, :])
            Wdiag = singles.tile([128, 128], mdt)
            cvec = singles.tile([128, 1], f32)
            ones_big = singles.tile([128, 256], mdt)
            identity = singles.tile([128, 128], mdt)
            nc.vector.tensor_copy(Wdiag[:], Wdiag_st[:])
            nc.vector.tensor_copy(identity[:], ident_st[:])
            nc.scalar.copy(ones_big[:], ones_st[:])
            nc.scalar.copy(cvec[:], cvec_st[:])

            x_view = x_dram.ap().rearrange("(t p j) d -> t p (j d)", t=T_TILES, p=128)
            out_view = out_dram.ap().rearrange("(t p f) -> t p f", t=T_TILES, p=128)

            # ---------------- main loop ----------------
            def main_body():
              sums_tiles = []
              for t in range(T_TILES):
                  x_sb = xin_pool.tile([128, 512 * D], f32)
                  if "xdma" in ablate:
                      nc.sync.dma_start(x_sb[:, 0:48], x_view[t][:, 0:48])
                  else:
                      nc.sync.dma_start(x_sb[:], x_view[t])

                  F = f_pool.tile([128, 512, 8], mdt)
                  xg = x_sb[:].rearrange("p (j d) -> p j d", d=D)
                  nc.vector.tensor_mul(F[:, :, 0:3], xg, xg)
                  nc.vector.tensor_copy(F[:, :, 3:6], xg)
                  # pad cols 6,7: any finite values (their Wdiag rows are 0)
                  nc.vector.tensor_copy(F[:, :, 6:8], xg[:, :, 0:2])
                  Fflat = F[:].rearrange("p j c -> p (j c)")

                  sums = spsum_pool.tile([128, 512], f32)
                  sums_tiles.append(sums)
                  dualexp = "nodual" not in ablate
                  if "pair" in ablate:
                      # paired batches: one [128,1024] tpsum + one wide ft-copy
                      for pair in range(4):
                          tp2 = tpsum_pool.tile([128, 1024], mdt, tag="tp2", bufs=1)
                          for v in range(8):
                              cn = 8 * pair + v
                              nc.tensor.transpose(
                                  tp2[:, 128 * v:128 * v + 128],
                                  Fflat[:, 128 * cn:128 * cn + 128],
                                  identity[:],
                              )
                          ft2 = ft_pool.tile([128, 1024], mdt, tag="ft2")
                          nc.vector.tensor_copy(ft2[:], tp2[:])
                          for sub in range(2):
                              m2 = mpsum_pool.tile([128, 1024], f32)
                              for half in range(2):
                                  rows = slice(64 * half, 64 * half + 64)
                                  nc.tensor.matmul(
                                      m2[:, 512 * half:512 * half + 512],
                                      Wdiag[rows, :],
                                      ft2[rows, 512 * sub:512 * sub + 512],
                                      start=True, stop=True)
                              e2 = e_pool.tile([128, 1024], mdt, tag="e2")
                              nc.scalar.activation(e2[:], m2[:], AF.Exp,
                                                   bias=cvec[:], scale=1.0)
                              for half in range(2):
                                  s = 2 * (2 * pair + sub) + half
                                  nc.tensor.matmul(
                                      sums[:],
                                      ones_big[:, 120 - 8 * s:248 - 8 * s],
                                      e2[:, 512 * half:512 * half + 512],
                                      start=(s == 0), stop=(s == 15))
                      continue_batches = []
                  else:
                      continue_batches = range(8)
                  for batch in continue_batches:
                      tp = tpsum_pool.tile([128, 512], mdt)
                      TW = 8 if "transpose" in ablate else 128
                      for u in range(4):
                          cn = 4 * batch + u
                          nc.tensor.transpose(
                              tp[:, 128 * u:128 * u + TW],
                              Fflat[:, 128 * cn:128 * cn + 128],
                              identity[:, 0:TW],
                          )
                      ft = ft_pool.tile([128, 512], mdt)
                      FW = 64 if "ftcopy" in ablate else 512
                      nc.vector.tensor_copy(ft[:, 0:FW], tp[:, 0:FW])
                      if dualexp:
                          # one wide m-psum (2 banks) + one exp for both halves
                          m2 = mpsum_pool.tile([128, 1024], f32)
                          for half in range(2):
                              rows = slice(64 * half, 64 * half + 64)
                              nc.tensor.matmul(
                                  m2[:, 512 * half:512 * half + 512],
                                  Wdiag[rows, :], ft[rows, :],
                                  start=True, stop=True)
                          e2 = e_pool.tile([128, 1024], mdt, tag="e2")
                          nc.scalar.activation(e2[:], m2[:], AF.Exp,
                                               bias=cvec[:], scale=1.0)
                          for half in range(2):
                              s = 2 * batch + half
                              nc.tensor.matmul(
                                  sums[:],
                                  ones_big[:, 120 - 8 * s:248 - 8 * s],
                                  e2[:, 512 * half:512 * half + 512],
                                  start=(s == 0), stop=(s == 15))
                          continue
                      for half in range(2):
                          rows = slice(64 * half, 64 * half + 64)
                          m_ps = mpsum_pool.tile([128, 512], f32)
                          MW = 8 if "mm" in ablate else 512
                          tpos = (64 * half, 0) if "tilepos" in ablate else None
                          nc.tensor.matmul(
                              m_ps[:, 0:MW], Wdiag[rows, :], ft[rows, 0:MW],
                              start=True, stop=True, tile_position=tpos)
                          e_sb = e_pool.tile([128, 512], mdt)
                          EW = 8 if "exp" in ablate else 512
                          efunc = AF.Copy if "expcopy" in ablate else AF.Exp
                          if "expcopy" in ablate:
                              nc.scalar.copy(e_sb[:, 0:EW], m_ps[:, 0:EW])
                          elif "nobias" in ablate:
                              nc.scalar.activation(e_sb[:, 0:EW], m_ps[:, 0:EW],
                                                   efunc)
                          elif "expf32" in ablate:
                              ef = e_pool.tile([128, 512], f32, tag="ef32")
                              nc.scalar.activation(ef[:, 0:EW], m_ps[:, 0:EW],
                                                   efunc, bias=cvec[:], scale=1.0)
                              nc.scalar.activation(e_sb[:, 0:8], m_ps[:, 0:8],
                                                   efunc, bias=cvec[:], scale=1.0)
                          else:
                              nc.scalar.activation(e_sb[:, 0:EW], m_ps[:, 0:EW],
                                                   efunc, bias=cvec[:], scale=1.0)
                          s = 2 * batch + half
                          OW = 8 if "ones" in ablate else 512
                          owin = 120 if "onesfix" in ablate else 120 - 8 * s
                          nc.tensor.matmul(
                              sums[:, 0:OW],
                              ones_big[:, owin:owin + 128],
                              e_sb[:, 0:OW],
                              start=(s == 0), stop=(s == 15))

                  if not defer_log:
                      out_sb = out_pool.tile([128, 512], f32)
                      nc.scalar.activation(out_sb[:], sums[:], AF.Ln)
                      nc.sync.dma_start(out_view[t], out_sb[:])

              if not defer_log:
                  sums_tiles = []   # logs already emitted inline
              # logs batched at the end (one act-table switch)
              for t in range(len(sums_tiles)):
                  out_sb = out_pool.tile([128, 512], f32)
                  nc.scalar.activation(out_sb[:], sums_tiles[t][:], AF.Ln)
                  if "odma" in ablate:
                      nc.sync.dma_start(out_view[t][:, 0:8], out_sb[:, 0:8])
                  else:
                      nc.sync.dma_start(out_view[t], out_sb[:])

            if reps == 1:
                main_body()
            else:
                with tc.For_i(0, reps, 1):
                    main_body()

    nc.compile()
    return nc


def _output_permutation():
    """n[l]: point index for each linear output position l (per core)."""
    tt, PP, ff = np.meshgrid(np.arange(T_TILES), np.arange(128), np.arange(512),
                             indexing="ij")
    batch, Pr = PP // 16, PP % 16
    half, b = Pr // 8, Pr % 8
    u, p = ff // 128, ff % 128
    n = (tt * 128 + p) * 512 + 64 * batch + 16 * u + 8 * half + b
    return n.reshape(-1)


def _host_constants(means, covariances, weights):
    """Wdiag [128,128], cvec [128,1], ones_big [128,256], identity [128,128]."""
    covp = covariances.astype(np.float64) + EPS
    mu = means.astype(np.float64)
    A = -0.5 / covp                              # [K,D] coeff of x^2
    B = mu / covp                                # [K,D] coeff of x
    c_k = (-0.5 * (mu * mu / covp).sum(1) - 0.5 * np.log(covp).sum(1)
           - 0.5 * D * np.log(2 * np.pi) + np.log(weights.astype(np.float64)))

    coefT = np.zeros((8, K), np.float32)
    coefT[0:3] = A.T
    coefT[3:6] = B.T
    wd8 = np.zeros((64, 128), np.float32)
    for b in range(8):
        wd8[8 * b:8 * b + 8, 16 * b:16 * b + 16] = coefT
    wdiag = np.concatenate([wd8, wd8], 0)

    cvec = np.tile(c_k.astype(np.float32), 8).reshape(128, 1)

    ones_big = np.zeros((128, 256), np.float32)
    for b in range(8):
        ones_big[16 * b:16 * b + 16, 120 + b] = 1.0

    ident = np.eye(128, dtype=np.float32)
    return wdiag, cvec, ones_big, ident


def kernel(x, means, covariances, weights):
    from concourse.bass_utils import run_bass_kernel_spmd

    x = np.ascontiguousarray(np.asarray(x, dtype=np.float32))
    means = np.ascontiguousarray(np.asarray(means, dtype=np.float32))
    covariances = np.ascontiguousarray(np.asarray(covariances, dtype=np.float32))
    weights = np.ascontiguousarray(np.asarray(weights, dtype=np.float32)).reshape(K)

    n = x.shape[0]
    x_pad = np.zeros((N_PAD, D), dtype=np.float32)
    x_pad[:n] = x

    key = "nc"
    if key not in _compiled_cache:
        _compiled_cache[key] = _build_nc(use_f32r=True)
    nc = _compiled_cache[key]

    wdiag, cvec, ones_big, ident = _host_constants(means, covariances, weights)

    in_maps = []
    for c in range(N_CORES):
        shard = x_pad[c * NPC:(c + 1) * NPC]
        in_maps.append({
            "x": np.ascontiguousarray(shard),
            "wdiag": wdiag,
            "cvec": cvec,
            "onesbig": ones_big,
            "ident": ident,
        })

    res = run_bass_kernel_spmd(
        nc, in_maps, core_ids=list(range(N_CORES)),
        trace=bool(int(os.environ.get("GMM_TRACE", "0"))),
    )
    kernel.last_results = res

    perm = _output_permutation()
    out_pad = np.empty(N_PAD, dtype=np.float32)
    for c in range(N_CORES):
        raw = res.results[c]["out"].reshape(-1)
        out_pad[c * NPC + perm] = raw
    return out_pad[:n]



# revision 11
# speedup vs baseline: 8.9850x; 8.9850x over previous
"""Trainium2 Bass kernel for DifferentiableGMM log-likelihood.

Computes  out[n] = logsumexp_k( -0.5*||(x[n]-mu[k])/s[k]||^2 - log|s[k]| + log w[k] )
for N=2,000,000 points, K=16 diagonal-covariance components, D=3.

V2 strategy (pure data parallel over 8 cores, 262144 points per core):
  lp[n,k] = sum_d A[k,d]*x[n,d]^2 + B[k,d]*x[n,d] + c_k  -- an 8-feature
  (6 real + 2 pad) contraction done on the PE in fp16 (host-verified
  max rel err 1.1e-3, 17x under the 2e-2 gate).  Features F=[x^2, x, pad]
  are built in fp16 (DVE), transposed on the PE at 1.0 cyc/col (fp16),
  copied PSUM->SBUF at DVE 2x_1P (fp16 dense), then contracted with the
  fp16 coefficient matrix (PE, 1 cyc/col).  exp() folds c_k via the
  per-partition activation bias (ACT, f32r out -- fp16 would flush the
  far tail to zero), the sum over k is a ones-matmul accumulating 16
  rounds into one [128,512] PSUM tile (PE), followed by Ln (ACT) and DMA
  out.  Device output order is interleaved; the host gathers it back.

Per-core engine budget (sim cost model):
  ACT exp+ln ~34us (the wall), PE ~34us, DVE ~26us, DMA ~13us.
"""

import os
import numpy as np

K = 16
D = 3
EPS = 1e-6
N_CORES = 8
N_FULL = 2_000_000

# per-core tiling
T_TILES = 4                      # x-tiles per core
TILE_PTS = 128 * 512             # points per x-tile
NPC = T_TILES * TILE_PTS         # 262144 points per core
N_PAD = N_CORES * NPC            # 2097152

_compiled_cache = {}


def _build_nc(use_f32r=True):
    # Force the ACT-table chooser to use the one set that holds Exp and Ln
    # together, so no table reloads happen mid-kernel.
    import concourse.bacc as _bacc_mod
    from concourse.hw_specs import get_activation_tables as _orig_gat
    def _only_combined(arch, __orig=_orig_gat):
        return {name: (fns if name == "natural_log_exp_and_others" else set())
                for name, fns in __orig(arch).items()}
    _bacc_mod.get_activation_tables = _only_combined
    reps = int(os.environ.get("GMM_REPS", "1"))
    fb_dve = bool(int(os.environ.get("GMM_FB_DVE", "0")))
    import concourse.bacc as bacc
    import concourse.mybir as mybir
    import concourse.tile as tile
    from concourse._compat import get_trn_type

    f32 = mybir.dt.float32
    f32r = mybir.dt.float32r
    f16 = mybir.dt.float16
    AF = mybir.ActivationFunctionType

    nc = bacc.Bacc(
        get_trn_type() or "TRN2",
        target_bir_lowering=False,
        debug=False,
        num_devices=N_CORES,
    )

    x_dram = nc.dram_tensor("x", [NPC, D], f32, kind="ExternalInput")
    wdiag_dram = nc.dram_tensor("wdiag", [128, 128], f16, kind="ExternalInput")
    cvec_dram = nc.dram_tensor("cvec", [128, 1], f32, kind="ExternalInput")
    ones_dram = nc.dram_tensor("onesbig", [128, 256], f32r, kind="ExternalInput")
    ident_dram = nc.dram_tensor("ident", [128, 128], f16, kind="ExternalInput")
    out_dram = nc.dram_tensor("out", [NPC], f32, kind="ExternalOutput")

    f_bufs = int(os.environ.get("GMM_F", "3"))

    with tile.TileContext(nc) as tc:
        with (
            tc.tile_pool(name="singles", bufs=1) as singles,
            tc.tile_pool(name="xin", bufs=int(os.environ.get("GMM_XIN", "3"))) as xin_pool,
            tc.tile_pool(name="f8", bufs=f_bufs) as f_pool,
            tc.tile_pool(name="ft", bufs=int(os.environ.get("GMM_FT", "3"))) as ft_pool,
            tc.tile_pool(name="etile", bufs=int(os.environ.get("GMM_E", "3"))) as e_pool,
            tc.tile_pool(name="osb", bufs=3) as out_pool,
            tc.tile_pool(name="tpsum", bufs=int(os.environ.get("GMM_TP", "2")), space="PSUM") as tpsum_pool,
            tc.tile_pool(name="mpsum", bufs=int(os.environ.get("GMM_MP", "2")), space="PSUM") as mpsum_pool,
            tc.tile_pool(name="spsum", bufs=int(os.environ.get("GMM_SP", "2")), space="PSUM") as spsum_pool,
        ):
            # Constants, staged through compute-engine copies so consumer
            # waits merge into their existing sem domains.
            Wd_st = singles.tile([128, 128], f16)
            cvec_st = singles.tile([128, 1], f32)
            ones_st = singles.tile([128, 256], f32r)
            ident_st = singles.tile([128, 128], f16)
            nc.sync.dma_start(Wd_st[:], wdiag_dram[:, :])
            nc.sync.dma_start(cvec_st[:], cvec_dram[:, :])
            nc.sync.dma_start(ones_st[:], ones_dram[:, :])
            nc.sync.dma_start(ident_st[:], ident_dram[:, :])
            Wd = singles.tile([128, 128], f16)
            cvec = singles.tile([128, 1], f32)
            ones_big = singles.tile([128, 256], f32r)
            identity = singles.tile([128, 128], f16)
            nc.vector.tensor_copy(Wd[:], Wd_st[:])
            nc.vector.tensor_copy(identity[:], ident_st[:])
            nc.scalar.copy(ones_big[:], ones_st[:])
            nc.scalar.copy(cvec[:], cvec_st[:])

            x_view = x_dram.ap().rearrange("(t p j) d -> t p (j d)", t=T_TILES, p=128)
            out_view = out_dram.ap().rearrange("(t p f) -> t p f", t=T_TILES, p=128)

            def main_body():
              # Software-pipelined over P = transpose-pair groups (2 g-groups
              # per pair, 4 pairs per x-tile).  Stage skew per iteration:
              #   T(P) | ft-copy/MM/exp(P-1) | ones(P-2)
              # keeps the in-order PE queue from round-tripping through DVE
              # (transpose->copy->matmul) or ACT (exp->ones) within a pair.
              NP = T_TILES * 4
              tp2s = {}
              ft2s = {}
              e2s = {}
              sums_t = {}

              def stage_T(P):
                  t = P // 4
                  if P % 4 == 0:
                      x_sb = xin_pool.tile([128, 512 * D], f32, tag="x")
                      nc.sync.dma_start(x_sb[:], x_view[t])
                      F = f_pool.tile([128, 512, 8], f16, tag="F")
                      xg = x_sb[:].rearrange("p (j d) -> p j d", d=D)
                      # F-build lives on gpsimd: on DVE it would block the
                      # ft copies in the in-order queue and stall the PE.
                      # pads must be finite (their Wd rows are 0, but NaN*0
                      # would poison the accumulate).
                      fb = nc.vector if fb_dve else nc.gpsimd
                      fb.tensor_mul(F[:, :, 0:3], xg, xg)
                      fb.tensor_copy(F[:, :, 3:6], xg)
                      nc.gpsimd.memset(F[:, :, 6:8], 1.0)
                      stage_T.Fflat = F[:].rearrange("p j c -> p (j c)")
                  Fflat = stage_T.Fflat
                  tp2 = tpsum_pool.tile([128, 1024], f16, tag="tp2")
                  tp2s[P] = tp2
                  for v in range(8):
                      c = 8 * (P % 4) + v
                      nc.tensor.transpose(
                          tp2[:, 128 * v:128 * v + 128],
                          Fflat[:, 128 * c:128 * c + 128],
                          identity[:],
                      )

              def stage_mid(P):
                  # ft copy + lp matmuls + exp for both g-groups of pair P
                  tp2 = tp2s.pop(P)
                  ft2 = ft_pool.tile([128, 1024], f16, tag="ft2")
                  ft2s[P] = ft2
                  nc.vector.tensor_copy(ft2[:], tp2[:])
                  t = P // 4
                  if P % 4 == 0:
                      sums_t[t] = spsum_pool.tile([128, 512], f32, tag="sums", name="sums")
                  e2_pair = []
                  for j in range(2):
                      m2 = mpsum_pool.tile([128, 1024], f32, tag="m2")
                      for h in range(2):
                          rows = slice(64 * h, 64 * h + 64)
                          nc.tensor.matmul(m2[:, 512 * h:512 * h + 512],
                                           Wd[rows, :],
                                           ft2[rows, 512 * j:512 * j + 512],
                                           start=True, stop=True)
                      e2 = e_pool.tile([128, 1024], f32r, tag="e2")
                      nc.scalar.activation(e2[:], m2[:], AF.Exp,
                                           bias=cvec[:], scale=1.0)
                      e2_pair.append(e2)
                  e2s[P] = e2_pair

              def stage_ones(P):
                  t = P // 4
                  sums = sums_t[t]
                  e2_pair = e2s.pop(P)
                  for j in range(2):
                      g = 2 * P + j
                      for h in range(2):
                          s = 2 * g + h
                          nc.tensor.matmul(
                              sums[:],
                              ones_big[:, 120 - 8 * (s % 16):248 - 8 * (s % 16)],
                              e2_pair[j][:, 512 * h:512 * h + 512],
                              start=(s % 16 == 0), stop=(s % 16 == 15))
                  if P % 4 == 3:
                      out_sb = out_pool.tile([128, 512], f32, tag="osb")
                      nc.scalar.activation(out_sb[:], sums_t.pop(t)[:], AF.Ln)
                      nc.sync.dma_start(out_view[t], out_sb[:])

              for P in range(NP + 2):
                  if P < NP:
                      stage_T(P)
                  if 1 <= P <= NP:
                      stage_mid(P - 1)
                  if P >= 2:
                      stage_ones(P - 2)

            unroll = int(os.environ.get("GMM_UNROLL", "1"))
            stag = bool(int(os.environ.get("GMM_STAG", "1")))
            if reps == 1:
                main_body()
            else:
                assert reps % unroll == 0 or reps == 1
                with tc.For_i(0, reps // unroll, 1, staggered_reset=stag):
                    for _ in range(unroll):
                        main_body()

    nc.compile()
    return nc


def _output_permutation():
    """n[l]: point index for each linear output position l (per core)."""
    tt, PP, ff = np.meshgrid(np.arange(T_TILES), np.arange(128), np.arange(512),
                             indexing="ij")
    g, Pr = PP // 16, PP % 16
    half, b = Pr // 8, Pr % 8
    u, p = ff // 128, ff % 128
    n = (tt * 128 + p) * 512 + 64 * g + 16 * u + 8 * half + b
    return n.reshape(-1)


def _host_constants(means, covariances, weights):
    """Wd [128,128] fp16, cvec [128,1], ones_big [128,256], ident16 [128,128]."""
    covp = covariances.astype(np.float64) + EPS
    mu = means.astype(np.float64)
    A = -0.5 / covp                              # [K,D] coeff of x^2
    B = mu / covp                                # [K,D] coeff of x
    c_k = (-0.5 * (mu * mu / covp).sum(1) - 0.5 * np.log(covp).sum(1)
           - 0.5 * D * np.log(2 * np.pi) + np.log(weights.astype(np.float64)))

    coefT = np.zeros((8, K), np.float32)
    coefT[0:3] = A.T
    coefT[3:6] = B.T
    wd8 = np.zeros((64, 128), np.float32)
    for b in range(8):
        wd8[8 * b:8 * b + 8, 16 * b:16 * b + 16] = coefT
    wdiag = np.concatenate([wd8, wd8], 0).astype(np.float16)

    cvec = np.tile(c_k.astype(np.float32), 8).reshape(128, 1)

    ones_big = np.zeros((128, 256), np.float32)
    for b in range(8):
        ones_big[16 * b:16 * b + 16, 120 + b] = 1.0

    ident = np.eye(128, dtype=np.float16)
    return wdiag, cvec, ones_big, ident


def kernel(x, means, covariances, weights):
    from concourse.bass_utils import run_bass_kernel_spmd

    x = np.ascontiguousarray(np.asarray(x, dtype=np.float32))
    means = np.ascontiguousarray(np.asarray(means, dtype=np.float32))
    covariances = np.ascontiguousarray(np.asarray(covariances, dtype=np.float32))
    weights = np.ascontiguousarray(np.asarray(weights, dtype=np.float32)).reshape(K)

    n = x.shape[0]
    x_pad = np.zeros((N_PAD, D), dtype=np.float32)
    x_pad[:n] = x

    key = "nc"
    if key not in _compiled_cache:
        _compiled_cache[key] = _build_nc(use_f32r=True)
    nc = _compiled_cache[key]

    wdiag, cvec, ones_big, ident = _host_constants(means, covariances, weights)

    in_maps = []
    for c in range(N_CORES):
        shard = x_pad[c * NPC:(c + 1) * NPC]
        in_maps.append({
            "x": np.ascontiguousarray(shard),
            "wdiag": wdiag,
            "cvec": cvec,
            "onesbig": ones_big,
            "ident": ident,
        })

    res = run_bass_kernel_spmd(
        nc, in_maps, core_ids=list(range(N_CORES)),
        trace=bool(int(os.environ.get("GMM_TRACE", "0"))),
    )
    kernel.last_results = res

    perm = _output_permutation()
    out_pad = np.empty(N_PAD, dtype=np.float32)
    for c in range(N_CORES):
        raw = res.results[c]["out"].reshape(-1)
        out_pad[c * NPC + perm] = raw
    return out_pad[:n]


# revision 16
# speedup vs baseline: 14.3673x; 1.5990x over previous
"""Trainium2 Bass kernel for DifferentiableGMM log-likelihood.

Computes  out[n] = logsumexp_k( -0.5*||(x[n]-mu[k])/s[k]||^2 - log|s[k]| + log w[k] )
for N=2,000,000 points, K=16 diagonal-covariance components, D=3.

V4 strategy (pure data parallel over 8 cores, 262144 points per core):
  lp[n,k] = sum_d A[k,d]*x[n,d]^2 + B[k,d]*x[n,d] + c_k  -- an 8-feature
  (6 real + 2 pad) contraction done on the PE in fp16 (host-verified max
  rel err ~1e-3, ~17x under the 2e-2 gate).

  HW is PE-instruction-count bound (sequencer + LDWEIGHTS overhead), so
  V4 removes the PE transpose stage entirely: features F=[x^2, x, 1, 1]
  are built in fp16 on gpsimd, then block-transposed SBUF->SBUF by the
  DVE stream transpose (independent 32x32 blocks).  The resulting layout
  puts (point-slot, feat) x 4 partition-bands on the contraction axis; a
  block-diagonal fp16 W (one 8x8 coef block per (band, point-slot))
  computes 8 components per matmul, two matmuls (comp halves lo/hi) per
  512-col chunk.  c_k is folded into W through the constant-1.0 pad
  feature (a single per-partition exp bias cannot serve both comp
  halves).  exp() outputs f32r (fp16 would flush the far tail to zero),
  the sum over k is a windowed ones-matmul accumulating 16 rounds (8
  chunks x 2 halves) into one [128,512] PSUM tile (PE), then Ln (ACT)
  and DMA out.  Device output order is interleaved; the host gathers it.

  Per-rep instruction budget: PE 128 MM + 64 LDW (was 256 MM + 192 LDW),
  DVE 8 stream transposes, ACT 32 exp + 4 ln, gpsimd 12 F-build ops.
  Sim per-core engine busy: ACT ~34us (wall), PE ~27us, DVE ~19us.
"""

import os
import numpy as np

K = 16
D = 3
EPS = 1e-6
N_CORES = 8
N_FULL = 2_000_000

# per-core tiling
T_TILES = 4                      # x-tiles per core
TILE_PTS = 128 * 512             # points per x-tile
NPC = T_TILES * TILE_PTS         # 262144 points per core
N_PAD = N_CORES * NPC            # 2097152

_compiled_cache = {}


def _build_nc(use_f32r=True):
    # Force the ACT-table chooser to use the one set that holds Exp and Ln
    # together, so no table reloads happen mid-kernel.
    import concourse.bacc as _bacc_mod
    from concourse.hw_specs import get_activation_tables as _orig_gat
    def _only_combined(arch, __orig=_orig_gat):
        return {name: (fns if name == "natural_log_exp_and_others" else set())
                for name, fns in __orig(arch).items()}
    _bacc_mod.get_activation_tables = _only_combined
    reps = int(os.environ.get("GMM_REPS", "1"))
    fb_dve = bool(int(os.environ.get("GMM_FB_DVE", "0")))
    ablate = set(os.environ.get("GMM_ABLATE", "").split(","))
    import concourse.bacc as bacc
    import concourse.mybir as mybir
    import concourse.tile as tile
    from concourse._compat import get_trn_type

    f32 = mybir.dt.float32
    f32r = mybir.dt.float32r
    f16 = mybir.dt.float16
    AF = mybir.ActivationFunctionType

    nc = bacc.Bacc(
        get_trn_type() or "TRN2",
        target_bir_lowering=False,
        debug=False,
        num_devices=N_CORES,
    )

    x_dram = nc.dram_tensor("x", [NPC, D], f32, kind="ExternalInput")
    wdiag_dram = nc.dram_tensor("wdiag", [128, 256], f16, kind="ExternalInput")
    cvec_dram = nc.dram_tensor("cvec", [128, 1], f32, kind="ExternalInput")
    ones_dram = nc.dram_tensor("onesbig", [128, 240], f32r, kind="ExternalInput")
    out_dram = nc.dram_tensor("out", [NPC], f32, kind="ExternalOutput")

    NCH = T_TILES * 8            # 512-col chunks per rep

    with tile.TileContext(nc) as tc:
        with (
            tc.tile_pool(name="singles", bufs=1) as singles,
            tc.tile_pool(name="xin", bufs=int(os.environ.get("GMM_XIN", "3"))) as xin_pool,
            tc.tile_pool(name="f8", bufs=int(os.environ.get("GMM_F", "3"))) as f_pool,
            tc.tile_pool(name="ftp", bufs=int(os.environ.get("GMM_FT", "2"))) as ft_pool,
            tc.tile_pool(name="etile", bufs=int(os.environ.get("GMM_E", "3"))) as e_pool,
            tc.tile_pool(name="osb", bufs=3) as out_pool,
            tc.tile_pool(name="mpsum", bufs=int(os.environ.get("GMM_MP", "3")), space="PSUM") as mpsum_pool,
            tc.tile_pool(name="spsum", bufs=int(os.environ.get("GMM_SP", "2")), space="PSUM") as spsum_pool,
        ):
            # Constants, staged through compute-engine copies so consumer
            # waits merge into their existing sem domains.
            Wd_st = singles.tile([128, 256], f16)
            cvec_st = singles.tile([128, 1], f32)
            ones_st = singles.tile([128, 240], f32r)
            nc.sync.dma_start(Wd_st[:], wdiag_dram[:, :])
            nc.sync.dma_start(cvec_st[:], cvec_dram[:, :])
            nc.sync.dma_start(ones_st[:], ones_dram[:, :])
            Wd = singles.tile([128, 256], f16)
            cvec = singles.tile([128, 1], f32)
            ones_big = singles.tile([128, 240], f32r)
            nc.vector.tensor_copy(Wd[:], Wd_st[:])
            nc.scalar.copy(ones_big[:], ones_st[:])
            nc.scalar.copy(cvec[:], cvec_st[:])

            x_view = x_dram.ap().rearrange("(t p j) d -> t p (j d)", t=T_TILES, p=128)
            out_view = out_dram.ap().rearrange("(t p f) -> t p f", t=T_TILES, p=128)

            def main_body():
              # Pipeline over C = 512-col chunks (8 per x-tile).  Stage skew:
              #   fetch/transpose | MM+exp(C-1) | ones(C-2)
              FTs = {}
              e2s = {}
              sums_t = {}
              Fs = {}

              def stage_fetch(t):
                  x_sb = xin_pool.tile([128, 512 * D], f32, tag="x")
                  nc.sync.dma_start(x_sb[:], x_view[t])
                  F = f_pool.tile([128, 512, 8], f16, tag="F")
                  xg = x_sb[:].rearrange("p (j d) -> p j d", d=D)
                  # F-build on gpsimd: on DVE it would delay the stream
                  # transposes.  Pad feats are memset to 1.0: feat 6 hits a
                  # zero W row (and must be finite -- NaN*0 would poison the
                  # accumulate), feat 7 carries c_k through W.
                  fb = nc.vector if fb_dve else nc.gpsimd
                  fb.tensor_mul(F[:, :, 0:3], xg, xg)
                  fb.tensor_copy(F[:, :, 3:6], xg)
                  nc.gpsimd.memset(F[:, :, 6:8], 1.0)
                  Fs[t] = F

              def stage_vt(C):
                  # one DVE stream-transpose per half x-tile (4 chunks)
                  t, half = C // 8, (C % 8) // 4
                  if half == 0:
                      FT = ft_pool.tile([128, 4096], f16, tag="FT")
                      FTs[t] = FT
                  FT = FTs[t]
                  Fflat = Fs[t][:].rearrange("p j c -> p (j c)")
                  nc.vector.transpose(FT[:, 2048 * half:2048 * half + 2048],
                                      Fflat[:, 2048 * half:2048 * half + 2048])

              def stage_mm(C):
                  t, c = C // 8, C % 8
                  FT = FTs[t]
                  if c == 0:
                      sums_t[t] = spsum_pool.tile([128, 512], f32, tag="sums",
                                                  name="sums")
                  m2 = mpsum_pool.tile([128, 1024], f32, tag="m2")
                  MW = 8 if "lp" in ablate else 512
                  for H in range(2):
                      nc.tensor.matmul(m2[:, 512 * H:512 * H + MW],
                                       Wd[:, 128 * H:128 * H + 128],
                                       FT[:, 512 * c:512 * c + MW],
                                       start=True, stop=True)
                  e2 = e_pool.tile([128, 1024], f32r, tag="e2")
                  EW = 64 if "exp" in ablate else 1024
                  nc.scalar.activation(e2[:, 0:EW], m2[:, 0:EW], AF.Exp,
                                       bias=0.0, scale=1.0)
                  e2s[C] = e2

              def stage_ones(C):
                  t, c = C // 8, C % 8
                  sums = sums_t[t]
                  e2 = e2s.pop(C)
                  OW = 8 if "ones" in ablate else 512
                  for H in range(2):
                      nc.tensor.matmul(
                          sums[:, 0:OW],
                          ones_big[:, 112 - 16 * c:240 - 16 * c],
                          e2[:, 512 * H:512 * H + OW],
                          start=(c == 0 and H == 0), stop=(c == 7 and H == 1))
                  if c == 7:
                      out_sb = out_pool.tile([128, 512], f32, tag="osb")
                      nc.scalar.activation(out_sb[:], sums_t.pop(t)[:], AF.Ln)
                      nc.sync.dma_start(out_view[t], out_sb[:])

              for C in range(-4, NCH + 2):
                  if C + 4 < NCH and (C + 4) % 8 == 0:
                      stage_fetch((C + 4) // 8)
                  if 0 <= C < NCH and C % 4 == 0:
                      stage_vt(C)
                  if 0 <= C - 1 < NCH:
                      stage_mm(C - 1)
                  if 0 <= C - 2 < NCH:
                      stage_ones(C - 2)

            unroll = int(os.environ.get("GMM_UNROLL", "1"))
            stag = bool(int(os.environ.get("GMM_STAG", "0")))
            if reps == 1:
                main_body()
            else:
                assert reps % unroll == 0 or reps == 1
                with tc.For_i(0, reps // unroll, 1, staggered_reset=stag):
                    for _ in range(unroll):
                        main_body()

    nc.compile()
    return nc


def _output_permutation():
    """n[l]: point index for each linear output position l (per core).

    sums partition P = 16c + 4a + p, free j: point has
      pgroup = 32a + (j % 32),  pt = 64c + 4*(j // 32) + p
    """
    tt, PP, ff = np.meshgrid(np.arange(T_TILES), np.arange(128), np.arange(512),
                             indexing="ij")
    c, Pr = PP // 16, PP % 16
    a, p = Pr // 4, Pr % 4
    jhi, jlo = ff // 32, ff % 32
    pgroup = 32 * a + jlo
    pt = 64 * c + 4 * jhi + p
    n = (tt * 128 + pgroup) * 512 + pt
    return n.reshape(-1)


def _host_constants(means, covariances, weights):
    """Wv [128,256] fp16 (lo|hi), cvec [128,1] (unused), ones_v [128,240]."""
    covp = covariances.astype(np.float64) + EPS
    mu = means.astype(np.float64)
    A = -0.5 / covp                              # [K,D] coeff of x^2
    B = mu / covp                                # [K,D] coeff of x
    c_k = (-0.5 * (mu * mu / covp).sum(1) - 0.5 * np.log(covp).sum(1)
           - 0.5 * D * np.log(2 * np.pi) + np.log(weights.astype(np.float64)))

    coefT = np.zeros((8, K), np.float64)
    coefT[0:3] = A.T
    coefT[3:6] = B.T
    coefT[7] = c_k                   # pad feature 7 is constant 1.0 in F
    # Wv[(32a + 8p + f), 128H + (32a + 8p + k')] = coefT[f, 8H + k']
    wv = np.zeros((128, 256), np.float64)
    for H in range(2):
        for a in range(4):
            for p in range(4):
                r = 32 * a + 8 * p
                wv[r:r + 8, 128 * H + r:128 * H + r + 8] = coefT[:, 8 * H:8 * H + 8]
    wv = wv.astype(np.float16)

    cvec = np.zeros((128, 1), np.float32)  # bias unused; c_k folded via pad

    # ones_v[(32a + 8p + k'), 112 + 4a + p] = 1 ; window 112-16c for chunk c
    ones_v = np.zeros((128, 240), np.float32)
    for a in range(4):
        for p in range(4):
            r = 32 * a + 8 * p
            ones_v[r:r + 8, 112 + 4 * a + p] = 1.0

    return wv, cvec, ones_v


def kernel(x, means, covariances, weights):
    from concourse.bass_utils import run_bass_kernel_spmd

    x = np.ascontiguousarray(np.asarray(x, dtype=np.float32))
    means = np.ascontiguousarray(np.asarray(means, dtype=np.float32))
    covariances = np.ascontiguousarray(np.asarray(covariances, dtype=np.float32))
    weights = np.ascontiguousarray(np.asarray(weights, dtype=np.float32)).reshape(K)

    n = x.shape[0]
    x_pad = np.zeros((N_PAD, D), dtype=np.float32)
    x_pad[:n] = x

    key = "nc"
    if key not in _compiled_cache:
        _compiled_cache[key] = _build_nc(use_f32r=True)
    nc = _compiled_cache[key]

    wdiag, cvec, ones_big = _host_constants(means, covariances, weights)

    in_maps = []
    for c in range(N_CORES):
        shard = x_pad[c * NPC:(c + 1) * NPC]
        in_maps.append({
            "x": np.ascontiguousarray(shard),
            "wdiag": wdiag,
            "cvec": cvec,
            "onesbig": ones_big,
        })

    res = run_bass_kernel_spmd(
        nc, in_maps, core_ids=list(range(N_CORES)),
        trace=bool(int(os.environ.get("GMM_TRACE", "0"))),
    )
    kernel.last_results = res

    perm = _output_permutation()
    out_pad = np.empty(N_PAD, dtype=np.float32)
    for c in range(N_CORES):
        raw = res.results[c]["out"].reshape(-1)
        out_pad[c * NPC + perm] = raw
    return out_pad[:n]
